# revision 20
# baseline (speedup 1.0000x reference)
"""Anchor-grid generation kernel for Trainium2 (8 NeuronCores, SPMD).

The reference computes RetinaNet-style anchors from the *shapes* of four FPN
feature maps — the feature values are never read.  Output is a tuple
(anchors_cxcywh, anchors_xyxy), each (783360, 4) float32.

Strategy: each anchor value decomposes as a rank-4 outer product over
(partition, free) once a level's per-core slice is laid out flat as
(128, N) on SBUF:

    tile[p, j] = 1*PHI[j] + cx0[p]*M0[j] + cy[p]*M1[j] + 1*WH[j]

which is exactly a K=4 fp32 matmul (lhsT = [1, cx0, cy, 1] per partition,
rhs = [PHI, M0, M1, WH]).  PE generates every output tile into PSUM,
ACT/DVE copy PSUM->SBUF, and DMA writes fully-contiguous DRAM ranges.
The 680 MiB of feature data never touches the device; per-core DRAM
traffic is ~3 MiB of pure output writes plus ~130 KB of seed constants.

All grid coordinates are exact fp32 integers, so the result is bit-exact
vs the fp32 reference (verified vs jax on CPU).
"""

import numpy as np

N_CORES = 8
# (H, W, stride, box_size) per pyramid level; hardcoded from the problem spec.
LEVELS = [(256, 256, 8, 32), (128, 128, 16, 64), (64, 64, 32, 128), (32, 32, 64, 256)]
NUM_ANCHORS = 9
TOTAL_ROWS = sum(H * W * NUM_ANCHORS for H, W, _, _ in LEVELS)  # 783360

# Per-level flat per-core layout: rows_per_core y-rows, g row-segments per
# y-row of Wseg grid columns each -> (128, N=Wseg*36) tile per core.
_LAYOUT = []
for H, W, S, B in LEVELS:
    rows_pc = H // N_CORES
    g = 128 // rows_pc
    Wseg = W // g
    _LAYOUT.append((H, W, S, B, rows_pc, g, Wseg, Wseg * 36))
# tensor names per (output, level)
_NAMES = [f"{t}{lvl}" for t in ("a", "x") for lvl in (3, 4, 5, 6)]


def _box_wh(box_size):
    # replicate reference._box_wh exactly (float64 math, fp32 cast at end)
    ratios = np.array([0.5, 1.0, 2.0])
    scales = np.array([2 ** 0.0, 2 ** (1.0 / 3.0), 2 ** (2.0 / 3.0)])
    scale_rep = np.tile(scales, len(ratios))
    ratio_rep = np.repeat(ratios, len(scales))
    side = box_size * scale_rep
    w = side * np.sqrt(ratio_rep)
    h = w / ratio_rep
    return np.stack([w, h], axis=-1).astype(np.float32)  # (9, 2)


def _host_consts():
    """K=5 fp16 seeds.  Every grid value (<=2044) is an exact fp16 integer;
    the irrational w/h templates are split hi+lo so the fp32 PSUM sum
    reconstructs them to ~1e-7 relative.

    Returns rhs_levels: per level a (5, 2*N) fp16 block [anchors | xyxy];
    lhst[c]: (5, 4*128) fp16 per core, rows = [1, cx0, cy, 1, 1]."""
    rhs_levels = []
    for H, W, S, B, rows_pc, g, Wseg, N in _LAYOUT:
        wh = _box_wh(B)
        a = np.arange(Wseg)
        k = np.arange(NUM_ANCHORS)
        base = (a[:, None] * 36 + k[None, :] * 4).ravel()  # (Wseg*9,)
        phi_v = np.repeat((S * a).astype(np.float32), NUM_ANCHORS)

        def hi_lo(full):
            hi = full.astype(np.float16)
            lo = (full - hi.astype(np.float32)).astype(np.float16)
            return hi, lo

        PHI = np.zeros(N, np.float16)
        M0 = np.zeros(N, np.float16)
        M1 = np.zeros(N, np.float16)
        WH = np.zeros(N, np.float32)
        PHI[base] = phi_v
        M0[base] = 1.0
        M1[base + 1] = 1.0
        WH[base + 2] = np.tile(wh[:, 0], Wseg)
        WH[base + 3] = np.tile(wh[:, 1], Wseg)
        WHhi, WHlo = hi_lo(WH)
        rhs_a = np.stack([PHI, M0, M1, WHhi, WHlo])

        PHIX = np.zeros(N, np.float16)
        M0X = np.zeros(N, np.float16)
        M1X = np.zeros(N, np.float16)
        WHX = np.zeros(N, np.float32)
        PHIX[base] = phi_v
        PHIX[base + 2] = phi_v
        M0X[base] = 1.0
        M0X[base + 2] = 1.0
        M1X[base + 1] = 1.0
        M1X[base + 3] = 1.0
        WHX[base] = np.tile(-wh[:, 0] / 2.0, Wseg)
        WHX[base + 1] = np.tile(-wh[:, 1] / 2.0, Wseg)
        WHX[base + 2] = np.tile(wh[:, 0] / 2.0, Wseg)
        WHX[base + 3] = np.tile(wh[:, 1] / 2.0, Wseg)
        WHXhi, WHXlo = hi_lo(WHX)
        rhs_x = np.stack([PHIX, M0X, M1X, WHXhi, WHXlo])
        rhs_levels.append(np.concatenate([rhs_a, rhs_x], axis=1))  # (5, 2N)

    lhst_per_core = []
    p = np.arange(128)
    for c in range(N_CORES):
        cols = []
        for H, W, S, B, rows_pc, g, Wseg, N in _LAYOUT:
            y = c * rows_pc + p // g
            cx0 = ((p % g) * Wseg + 0.5) * S
            cy = (y + 0.5) * S
            ones = np.ones(128, np.float16)
            cols.append(
                np.stack(
                    [ones, cx0.astype(np.float16), cy.astype(np.float16),
                     ones, ones]
                )
            )
        lhst_per_core.append(np.concatenate(cols, axis=1))  # (5, 512)
    return rhs_levels, lhst_per_core


def _build():
    """Raw bacc kernel (no TileContext): explicit engine blocks + semaphores.
    Avoids Tile's multi-microsecond end-of-kernel drain/barrier cascade.

    Dataflow per core: 1 input DMA (66 KB fp16 seed blob) -> 18 K=5 fp16
    matmuls into 8 PSUM banks -> PSUM->SBUF copies alternating DVE/ACT ->
    18 per-chunk HWDGE output DMAs (~3 MiB, the write roofline)."""
    import concourse.bacc as bacc
    import concourse.mybir as mybir

    f32 = mybir.dt.float32
    f16 = mybir.dt.float16
    rhs_np, lhst_np = _host_consts()

    # consts blob: [lhst(512) | r3 | r4 | r5 | r6] along free dim, 5 rows
    level_off = []
    off = 4 * 128
    for li in range(4):
        level_off.append(off)
        off += rhs_np[li].shape[1]
    blob_w = off  # 6632

    # chunk plan: (name, level, c0, c1, blob col offset).  The very first
    # chunk is 256 wide so the first PSUM->SBUF copy (and with it the output
    # DMA stream) starts as early as possible.
    chunks = []
    for li, (H, W, S, B, rows_pc, g, Wseg, N) in enumerate(_LAYOUT):
        for ti, t in enumerate(("a", "x")):
            if li == 0 and ti == 0:
                bounds = [0, 256, 768, 1280, 1792, 2304]
            else:
                bounds = list(range(0, N, 512)) + [N]
            for c0, c1 in zip(bounds[:-1], bounds[1:]):
                chunks.append(
                    (f"{t}{li + 3}", li, c0, c1, level_off[li] + ti * N + c0)
                )
    n_chunks = len(chunks)  # 18

    nc = bacc.Bacc("TRN2", target_bir_lowering=False, debug=False,
                   num_devices=N_CORES)

    # One fused output: columns [a3|x3|a4|x4|a5|x5|a6|x6] in chunk order —
    # both the SBUF staging tensor and the DRAM output share this layout, so
    # any run of consecutive chunks is a single rectangular DMA.
    out_w = sum(c1 - c0 for (_, _, c0, c1, _) in chunks)  # 6120
    chunk_col = []
    _cc = 0
    for _, _, c0, c1, _ in chunks:
        chunk_col.append(_cc)
        _cc += c1 - c0

    consts_in = nc.dram_tensor("consts", [5, blob_w], f16, kind="ExternalInput")
    out_dram = nc.dram_tensor("out", [128, out_w], f32, kind="ExternalOutput")

    consts = nc.alloc_sbuf_tensor("consts_sb", [5, blob_w], f16)
    stage = nc.alloc_sbuf_tensor("stage", [128, out_w], f32)
    psum = [nc.alloc_psum_tensor(f"ps{b}", [128, 512], f32) for b in range(8)]

    # cumulative per-copy-engine counts after chunk j (engine = j % 2)
    cnt_at = []
    _c = [0, 0]
    for j in range(n_chunks):
        _c[j % 2] += 1
        cnt_at.append(tuple(_c))

    # output DMA batches (by chunk index): uniform 2-chunk batches — each
    # ~512KB batch buys ~1.2us of stream for a ~0.7us issue, so the SP issue
    # loop stays ahead and the SDMA engines never run dry
    batches = [(j, min(j + 1, n_chunks - 1)) for j in range(0, n_chunks, 2)]

    with (
        nc.Block() as block,
        nc.semaphore("in_sem") as in_sem,
        nc.semaphore("in_sem2") as in_sem2,
        nc.semaphore("mm_sem") as mm_sem,
        nc.semaphore("cp0") as cp0,
        nc.semaphore("cp1") as cp1,
        nc.semaphore("od") as od,
    ):
        cp = (cp0, cp1)

        @block.sync
        def _(sync):
            # split input load: the first 1792 cols (lhsT + rhs for chunks
            # 0-2) land first so MM0 starts one small-DMA-receipt earlier;
            # part 2's receipt latency hides behind chunks 0-2
            sync.dma_start(
                consts[:, 0:1792], consts_in[:, 0:1792]
            ).then_inc(in_sem, 16)
            sync.dma_start(
                consts[:, 1792:blob_w], consts_in[:, 1792:blob_w]
            ).then_inc(in_sem2, 16)
            for jlo, jhi in batches:
                sync.wait_ge(cp0, cnt_at[jhi][0])
                if cnt_at[jhi][1]:
                    sync.wait_ge(cp1, cnt_at[jhi][1])
                lo = chunk_col[jlo]
                hi = chunk_col[jhi] + (chunks[jhi][3] - chunks[jhi][2])
                sync.dma_start(
                    out_dram[:, lo:hi], stage[:, lo:hi]
                ).then_inc(od, 16)
            if not _SKIP_OD_WAIT:
                sync.wait_ge(od, 16 * len(batches))

        @block.tensor
        def _(pe):
            pe.wait_ge(in_sem, 16)
            for j, (name, li, c0, c1, bcol) in enumerate(chunks):
                if j == 3:
                    pe.wait_ge(in_sem2, 16)
                if j >= 8:
                    f = j - 8  # chunk whose copy frees this bank
                    pe.wait_ge(cp[f % 2], cnt_at[f][f % 2])
                n = c1 - c0
                pe.matmul(
                    psum[j % 8][:, :n],
                    consts[:, li * 128 : (li + 1) * 128],
                    consts[:, bcol : bcol + n],
                    start=True,
                    stop=True,
                ).then_inc(mm_sem, 1)

        @block.vector
        def _(dve):
            for j, (name, li, c0, c1, _) in enumerate(chunks):
                if j % 2 != 0:
                    continue
                dve.wait_ge(mm_sem, j + 1)
                n = c1 - c0
                col = chunk_col[j]
                dve.tensor_copy(
                    stage[:, col : col + n], psum[j % 8][:, :n]
                ).then_inc(cp0, 1)

        @block.scalar
        def _(act):
            for j, (name, li, c0, c1, _) in enumerate(chunks):
                if j % 2 != 1:
                    continue
                act.wait_ge(mm_sem, j + 1)
                n = c1 - c0
                col = chunk_col[j]
                act.copy(stage[:, col : col + n], psum[j % 8][:, :n]).then_inc(
                    cp1, 1
                )

    nc.compile()
    blob = [
        np.concatenate([lhst_np[c]] + rhs_np, axis=1).astype(np.float16)
        for c in range(N_CORES)
    ]
    in_maps = [{"consts": blob[c]} for c in range(N_CORES)]
    return nc, in_maps


def _build_tile():
    import concourse.bass as bass
    import concourse.bacc as bacc
    import concourse.mybir as mybir
    from concourse import tile

    f32 = mybir.dt.float32
    f16 = mybir.dt.float16
    rhs_np, lhst_np = _host_consts()

    nc = bacc.Bacc("TRN2", target_bir_lowering=False, debug=False,
                   num_devices=N_CORES)

    lhst_in = nc.dram_tensor("lhst", [5, 4 * 128], f16, kind="ExternalInput")
    rhs_in = [
        nc.dram_tensor(f"r{li + 3}", list(rhs_np[li].shape), f16,
                       kind="ExternalInput")
        for li in range(4)
    ]
    outs = {}
    for t in ("a", "x"):
        for li, (H, W, S, B, rows_pc, g, Wseg, N) in enumerate(_LAYOUT):
            outs[f"{t}{li + 3}"] = nc.dram_tensor(
                f"{t}{li + 3}", [128, N], f32, kind="ExternalOutput"
            )

    with tile.TileContext(nc) as tc:
        with (
            tc.tile_pool(name="consts", bufs=1) as cpool,
            tc.tile_pool(name="stage", bufs=1) as spool,
            tc.tile_pool(name="psum", bufs=8, space="PSUM") as ppool,
        ):
            lhst = cpool.tile([5, 4 * 128], f16)
            nc.sync.dma_start(lhst[:], lhst_in[:])
            rhs = []
            for li in range(4):
                r = cpool.tile(list(rhs_np[li].shape), f16, tag=f"rhs{li}")
                nc.sync.dma_start(r[:], rhs_in[li][:])
                rhs.append(r)

            copy_i = 0
            for li, (H, W, S, B, rows_pc, g, Wseg, N) in enumerate(_LAYOUT):
                lhsT = lhst[:, li * 128 : (li + 1) * 128]
                for ti, t in enumerate(("a", "x")):
                    name = f"{t}{li + 3}"
                    out_t = spool.tile([128, N], f32, tag=name)
                    for c0 in range(0, N, 512):
                        c1 = min(c0 + 512, N)
                        ps = ppool.tile([128, c1 - c0], f32)
                        nc.tensor.matmul(
                            ps[:],
                            lhsT,
                            rhs[li][:, ti * N + c0 : ti * N + c1],
                            start=True,
                            stop=True,
                        )
                        if copy_i % 2 == 0:
                            nc.vector.tensor_copy(out_t[:, c0:c1], ps[:])
                        else:
                            nc.scalar.copy(out_t[:, c0:c1], ps[:])
                        copy_i += 1
                    nc.sync.dma_start(outs[name][:], out_t[:])

    nc.compile()
    in_maps = [
        {"lhst": lhst_np[c], **{f"r{li + 3}": rhs_np[li] for li in range(4)}}
        for c in range(N_CORES)
    ]
    return nc, in_maps


# Rely on the NEFF epilogue's DMA-queue quiesce instead of an explicit
# completion wait on the output DMAs (saves the ~1.7us HBM-write sem receipt
# and lets the fixed epilogue overlap the stream tail).
_SKIP_OD_WAIT = True

_CACHE = None


def _run(trace=False, **trace_kw):
    global _CACHE
    if _CACHE is None:
        _CACHE = _build()
    nc, in_maps = _CACHE
    from concourse.bass_utils import run_bass_kernel_spmd

    return run_bass_kernel_spmd(
        nc, in_maps, core_ids=list(range(N_CORES)), trace=trace, **trace_kw
    )


def _assemble(results):
    """results[c]["out"] is (128, 6120) fused as [a3|x3|a4|x4|a5|x5|a6|x6]."""
    anchors = np.empty((TOTAL_ROWS, 4), np.float32)
    xyxy = np.empty((TOTAL_ROWS, 4), np.float32)
    row_off = 0
    col = 0
    for li, (H, W, S, B, rows_pc, g, Wseg, N) in enumerate(_LAYOUT):
        rows_core = 128 * N // 4
        for c in range(N_CORES):
            lo = row_off + c * rows_core
            arr = results[c]["out"]
            anchors[lo : lo + rows_core] = arr[:, col : col + N].reshape(-1, 4)
            xyxy[lo : lo + rows_core] = arr[:, col + N : col + 2 * N].reshape(
                -1, 4
            )
        col += 2 * N
        row_off += H * W * NUM_ANCHORS
    return anchors, xyxy


def kernel(feat3=None, feat4=None, feat5=None, feat6=None, **_ignored):
    # Outputs depend only on the (fixed) feature shapes; values are unused.
    res = _run(trace=False)
    return _assemble(res.results)


def _ensure_ntff_hook():
    """Inject antenv.axon_hooks (absent in this image) so that
    run_bass_kernel_spmd(trace=True) can reach the ctypes NTFF profiler."""
    import sys
    import types

    try:
        from antenv.axon_hooks import get_axon_ntff_profile_hook  # noqa: F401

        return
    except ImportError:
        pass
    import antenv
    from trn_agent_boot.trn_boot import _ntff_profile_via_ctypes

    hook = _ntff_profile_via_ctypes("/opt/axon/libaxon_pjrt.so")
    mod = types.ModuleType("antenv.axon_hooks")
    mod._hook = hook
    mod.set_axon_ntff_profile_hook = lambda h: setattr(mod, "_hook", h)
    mod.get_axon_ntff_profile_hook = lambda: mod._hook
    sys.modules["antenv.axon_hooks"] = mod
    antenv.axon_hooks = mod


def kernel_traced(**trace_kw):
    """Run with NTFF profiling; returns ((anchors, xyxy), BassKernelResults)."""
    _ensure_ntff_hook()
    from concourse import bass_utils

    bass_utils.upload_artifacts = lambda tmpdir: tmpdir  # no egress in sandbox
    res = _run(trace=True, **trace_kw)
    return _assemble(res.results), res


# revision 23
# speedup vs baseline: 1.0606x; 1.0606x over previous
"""Anchor-grid generation kernel for Trainium2 (8 NeuronCores, SPMD).

The reference computes RetinaNet-style anchors from the *shapes* of four FPN
feature maps — the feature values are never read.  Output is a tuple
(anchors_cxcywh, anchors_xyxy), each (783360, 4) float32.

Strategy: each anchor value decomposes as a rank-4 outer product over
(partition, free) once a level's per-core slice is laid out flat as
(128, N) on SBUF:

    tile[p, j] = 1*PHI[j] + cx0[p]*M0[j] + cy[p]*M1[j] + 1*WH[j]

which is exactly a K=4 fp32 matmul (lhsT = [1, cx0, cy, 1] per partition,
rhs = [PHI, M0, M1, WH]).  PE generates every output tile into PSUM,
ACT/DVE copy PSUM->SBUF, and DMA writes fully-contiguous DRAM ranges.
The 680 MiB of feature data never touches the device; per-core DRAM
traffic is ~3 MiB of pure output writes plus ~130 KB of seed constants.

All grid coordinates are exact fp32 integers, so the result is bit-exact
vs the fp32 reference (verified vs jax on CPU).
"""

import numpy as np

N_CORES = 8
# (H, W, stride, box_size) per pyramid level; hardcoded from the problem spec.
LEVELS = [(256, 256, 8, 32), (128, 128, 16, 64), (64, 64, 32, 128), (32, 32, 64, 256)]
NUM_ANCHORS = 9
TOTAL_ROWS = sum(H * W * NUM_ANCHORS for H, W, _, _ in LEVELS)  # 783360

# Per-level flat per-core layout: rows_per_core y-rows, g row-segments per
# y-row of Wseg grid columns each -> (128, N=Wseg*36) tile per core.
_LAYOUT = []
for H, W, S, B in LEVELS:
    rows_pc = H // N_CORES
    g = 128 // rows_pc
    Wseg = W // g
    _LAYOUT.append((H, W, S, B, rows_pc, g, Wseg, Wseg * 36))
# tensor names per (output, level)
_NAMES = [f"{t}{lvl}" for t in ("a", "x") for lvl in (3, 4, 5, 6)]


def _box_wh(box_size):
    # replicate reference._box_wh exactly (float64 math, fp32 cast at end)
    ratios = np.array([0.5, 1.0, 2.0])
    scales = np.array([2 ** 0.0, 2 ** (1.0 / 3.0), 2 ** (2.0 / 3.0)])
    scale_rep = np.tile(scales, len(ratios))
    ratio_rep = np.repeat(ratios, len(scales))
    side = box_size * scale_rep
    w = side * np.sqrt(ratio_rep)
    h = w / ratio_rep
    return np.stack([w, h], axis=-1).astype(np.float32)  # (9, 2)


def _host_consts():
    """K=5 fp16 seeds.  Every grid value (<=2044) is an exact fp16 integer;
    the irrational w/h templates are split hi+lo so the fp32 PSUM sum
    reconstructs them to ~1e-7 relative.

    Returns rhs_levels: per level a (5, 2*N) fp16 block [anchors | xyxy];
    lhst[c]: (5, 4*128) fp16 per core, rows = [1, cx0, cy, 1, 1]."""
    rhs_levels = []
    for H, W, S, B, rows_pc, g, Wseg, N in _LAYOUT:
        wh = _box_wh(B)
        a = np.arange(Wseg)
        k = np.arange(NUM_ANCHORS)
        base = (a[:, None] * 36 + k[None, :] * 4).ravel()  # (Wseg*9,)
        phi_v = np.repeat((S * a).astype(np.float32), NUM_ANCHORS)

        def hi_lo(full):
            hi = full.astype(np.float16)
            lo = (full - hi.astype(np.float32)).astype(np.float16)
            return hi, lo

        PHI = np.zeros(N, np.float16)
        M0 = np.zeros(N, np.float16)
        M1 = np.zeros(N, np.float16)
        WH = np.zeros(N, np.float32)
        PHI[base] = phi_v
        M0[base] = 1.0
        M1[base + 1] = 1.0
        WH[base + 2] = np.tile(wh[:, 0], Wseg)
        WH[base + 3] = np.tile(wh[:, 1], Wseg)
        WHhi, WHlo = hi_lo(WH)
        rhs_a = np.stack([PHI, M0, M1, WHhi, WHlo])

        PHIX = np.zeros(N, np.float16)
        M0X = np.zeros(N, np.float16)
        M1X = np.zeros(N, np.float16)
        WHX = np.zeros(N, np.float32)
        PHIX[base] = phi_v
        PHIX[base + 2] = phi_v
        M0X[base] = 1.0
        M0X[base + 2] = 1.0
        M1X[base + 1] = 1.0
        M1X[base + 3] = 1.0
        WHX[base] = np.tile(-wh[:, 0] / 2.0, Wseg)
        WHX[base + 1] = np.tile(-wh[:, 1] / 2.0, Wseg)
        WHX[base + 2] = np.tile(wh[:, 0] / 2.0, Wseg)
        WHX[base + 3] = np.tile(wh[:, 1] / 2.0, Wseg)
        WHXhi, WHXlo = hi_lo(WHX)
        rhs_x = np.stack([PHIX, M0X, M1X, WHXhi, WHXlo])
        rhs_levels.append(np.concatenate([rhs_a, rhs_x], axis=1))  # (5, 2N)

    lhst_per_core = []
    p = np.arange(128)
    for c in range(N_CORES):
        cols = []
        for H, W, S, B, rows_pc, g, Wseg, N in _LAYOUT:
            y = c * rows_pc + p // g
            cx0 = ((p % g) * Wseg + 0.5) * S
            cy = (y + 0.5) * S
            ones = np.ones(128, np.float16)
            cols.append(
                np.stack(
                    [ones, cx0.astype(np.float16), cy.astype(np.float16),
                     ones, ones]
                )
            )
        lhst_per_core.append(np.concatenate(cols, axis=1))  # (5, 512)
    return rhs_levels, lhst_per_core


def _build():
    """Raw bacc kernel (no TileContext): explicit engine blocks + semaphores.
    Avoids Tile's multi-microsecond end-of-kernel drain/barrier cascade.

    Dataflow per core: 1 input DMA (66 KB fp16 seed blob) -> 18 K=5 fp16
    matmuls into 8 PSUM banks -> PSUM->SBUF copies alternating DVE/ACT ->
    18 per-chunk HWDGE output DMAs (~3 MiB, the write roofline)."""
    import concourse.bacc as bacc
    import concourse.mybir as mybir

    f32 = mybir.dt.float32
    f16 = mybir.dt.float16
    rhs_np, lhst_np = _host_consts()

    # consts blob: [lhst(512) | r3 | r4 | r5 | r6] along free dim, 5 rows
    level_off = []
    off = 4 * 128
    for li in range(4):
        level_off.append(off)
        off += rhs_np[li].shape[1]
    blob_w = off  # 6632

    # chunk plan: (name, level, c0, c1, blob col offset).  The very first
    # chunk is 256 wide so the first PSUM->SBUF copy (and with it the output
    # DMA stream) starts as early as possible.
    chunks = []
    for li, (H, W, S, B, rows_pc, g, Wseg, N) in enumerate(_LAYOUT):
        for ti, t in enumerate(("a", "x")):
            if li == 0 and ti == 0:
                bounds = [0, 256, 768, 1280, 1792, 2304]
            else:
                bounds = list(range(0, N, 512)) + [N]
            for c0, c1 in zip(bounds[:-1], bounds[1:]):
                chunks.append(
                    (f"{t}{li + 3}", li, c0, c1, level_off[li] + ti * N + c0)
                )
    n_chunks = len(chunks)  # 18

    nc = bacc.Bacc("TRN2", target_bir_lowering=False, debug=False,
                   num_devices=N_CORES)

    # One fused output: columns [a3|x3|a4|x4|a5|x5|a6|x6] in chunk order —
    # both the SBUF staging tensor and the DRAM output share this layout, so
    # any run of consecutive chunks is a single rectangular DMA.
    out_w = sum(c1 - c0 for (_, _, c0, c1, _) in chunks)  # 6120
    chunk_col = []
    _cc = 0
    for _, _, c0, c1, _ in chunks:
        chunk_col.append(_cc)
        _cc += c1 - c0

    consts_in = nc.dram_tensor("consts", [5, blob_w], f16, kind="ExternalInput")
    out_dram = nc.dram_tensor("out", [128, out_w], f32, kind="ExternalOutput")

    consts = nc.alloc_sbuf_tensor("consts_sb", [5, blob_w], f16)
    stage = nc.alloc_sbuf_tensor("stage", [128, out_w], f32)
    psum = [nc.alloc_psum_tensor(f"ps{b}", [128, 512], f32) for b in range(8)]

    # cumulative per-copy-engine counts after chunk j (engine = j % 2)
    cnt_at = []
    _c = [0, 0]
    for j in range(n_chunks):
        _c[j % 2] += 1
        cnt_at.append(tuple(_c))

    # output DMA batches (by chunk index).  Fewer DMA instructions is
    # faster in itself (the NEFF teardown costs ~0.8us per DMA), but each
    # batch can only issue after its last chunk is copied, so front-load:
    # tiny first batch for an early stream start, big middle, small tail.
    batches = [(0, 0), (1, 4), (5, 9), (10, 13), (14, 17)]

    with (
        nc.Block() as block,
        nc.semaphore("in_sem") as in_sem,
        nc.semaphore("in_sem2") as in_sem2,
        nc.semaphore("mm_sem") as mm_sem,
        nc.semaphore("cp0") as cp0,
        nc.semaphore("cp1") as cp1,
        nc.semaphore("od") as od,
    ):
        cp = (cp0, cp1)

        @block.sync
        def _(sync):
            # split input load: the first 1792 cols (lhsT + rhs for chunks
            # 0-2) land first so MM0 starts one small-DMA-receipt earlier;
            # part 2's receipt latency hides behind chunks 0-2
            sync.dma_start(
                consts[:, 0:1792], consts_in[:, 0:1792]
            ).then_inc(in_sem, 16)
            sync.dma_start(
                consts[:, 1792:blob_w], consts_in[:, 1792:blob_w]
            ).then_inc(in_sem2, 16)
            for jlo, jhi in batches:
                sync.wait_ge(cp0, cnt_at[jhi][0])
                if cnt_at[jhi][1]:
                    sync.wait_ge(cp1, cnt_at[jhi][1])
                lo = chunk_col[jlo]
                hi = chunk_col[jhi] + (chunks[jhi][3] - chunks[jhi][2])
                sync.dma_start(out_dram[:, lo:hi], stage[:, lo:hi]).then_inc(
                    od, 16
                )
            if not _SKIP_OD_WAIT:
                sync.wait_ge(od, 16 * len(batches))

        @block.tensor
        def _(pe):
            pe.wait_ge(in_sem, 16)
            for j, (name, li, c0, c1, bcol) in enumerate(chunks):
                if j == 3:
                    pe.wait_ge(in_sem2, 16)
                if j >= 8:
                    f = j - 8  # chunk whose copy frees this bank
                    pe.wait_ge(cp[f % 2], cnt_at[f][f % 2])
                n = c1 - c0
                pe.matmul(
                    psum[j % 8][:, :n],
                    consts[:, li * 128 : (li + 1) * 128],
                    consts[:, bcol : bcol + n],
                    start=True,
                    stop=True,
                ).then_inc(mm_sem, 1)

        @block.vector
        def _(dve):
            for j, (name, li, c0, c1, _) in enumerate(chunks):
                if j % 2 != 0:
                    continue
                dve.wait_ge(mm_sem, j + 1)
                n = c1 - c0
                col = chunk_col[j]
                dve.tensor_copy(
                    stage[:, col : col + n], psum[j % 8][:, :n]
                ).then_inc(cp0, 1)

        @block.scalar
        def _(act):
            for j, (name, li, c0, c1, _) in enumerate(chunks):
                if j % 2 != 1:
                    continue
                act.wait_ge(mm_sem, j + 1)
                n = c1 - c0
                col = chunk_col[j]
                act.copy(stage[:, col : col + n], psum[j % 8][:, :n]).then_inc(
                    cp1, 1
                )

    nc.compile()
    blob = [
        np.concatenate([lhst_np[c]] + rhs_np, axis=1).astype(np.float16)
        for c in range(N_CORES)
    ]
    in_maps = [{"consts": blob[c]} for c in range(N_CORES)]
    return nc, in_maps


def _build_tile():
    import concourse.bass as bass
    import concourse.bacc as bacc
    import concourse.mybir as mybir
    from concourse import tile

    f32 = mybir.dt.float32
    f16 = mybir.dt.float16
    rhs_np, lhst_np = _host_consts()

    nc = bacc.Bacc("TRN2", target_bir_lowering=False, debug=False,
                   num_devices=N_CORES)

    lhst_in = nc.dram_tensor("lhst", [5, 4 * 128], f16, kind="ExternalInput")
    rhs_in = [
        nc.dram_tensor(f"r{li + 3}", list(rhs_np[li].shape), f16,
                       kind="ExternalInput")
        for li in range(4)
    ]
    outs = {}
    for t in ("a", "x"):
        for li, (H, W, S, B, rows_pc, g, Wseg, N) in enumerate(_LAYOUT):
            outs[f"{t}{li + 3}"] = nc.dram_tensor(
                f"{t}{li + 3}", [128, N], f32, kind="ExternalOutput"
            )

    with tile.TileContext(nc) as tc:
        with (
            tc.tile_pool(name="consts", bufs=1) as cpool,
            tc.tile_pool(name="stage", bufs=1) as spool,
            tc.tile_pool(name="psum", bufs=8, space="PSUM") as ppool,
        ):
            lhst = cpool.tile([5, 4 * 128], f16)
            nc.sync.dma_start(lhst[:], lhst_in[:])
            rhs = []
            for li in range(4):
                r = cpool.tile(list(rhs_np[li].shape), f16, tag=f"rhs{li}")
                nc.sync.dma_start(r[:], rhs_in[li][:])
                rhs.append(r)

            copy_i = 0
            for li, (H, W, S, B, rows_pc, g, Wseg, N) in enumerate(_LAYOUT):
                lhsT = lhst[:, li * 128 : (li + 1) * 128]
                for ti, t in enumerate(("a", "x")):
                    name = f"{t}{li + 3}"
                    out_t = spool.tile([128, N], f32, tag=name)
                    for c0 in range(0, N, 512):
                        c1 = min(c0 + 512, N)
                        ps = ppool.tile([128, c1 - c0], f32)
                        nc.tensor.matmul(
                            ps[:],
                            lhsT,
                            rhs[li][:, ti * N + c0 : ti * N + c1],
                            start=True,
                            stop=True,
                        )
                        if copy_i % 2 == 0:
                            nc.vector.tensor_copy(out_t[:, c0:c1], ps[:])
                        else:
                            nc.scalar.copy(out_t[:, c0:c1], ps[:])
                        copy_i += 1
                    nc.sync.dma_start(outs[name][:], out_t[:])

    nc.compile()
    in_maps = [
        {"lhst": lhst_np[c], **{f"r{li + 3}": rhs_np[li] for li in range(4)}}
        for c in range(N_CORES)
    ]
    return nc, in_maps


# Rely on the NEFF epilogue's DMA-queue quiesce instead of an explicit
# completion wait on the output DMAs (saves the ~1.7us HBM-write sem receipt
# and lets the fixed epilogue overlap the stream tail).
_SKIP_OD_WAIT = True

_CACHE = None


def _run(trace=False, **trace_kw):
    global _CACHE
    if _CACHE is None:
        _CACHE = _build()
    nc, in_maps = _CACHE
    from concourse.bass_utils import run_bass_kernel_spmd

    return run_bass_kernel_spmd(
        nc, in_maps, core_ids=list(range(N_CORES)), trace=trace, **trace_kw
    )


def _assemble(results):
    """results[c]["out"] is (128, 6120) fused as [a3|x3|a4|x4|a5|x5|a6|x6]."""
    anchors = np.empty((TOTAL_ROWS, 4), np.float32)
    xyxy = np.empty((TOTAL_ROWS, 4), np.float32)
    row_off = 0
    col = 0
    for li, (H, W, S, B, rows_pc, g, Wseg, N) in enumerate(_LAYOUT):
        rows_core = 128 * N // 4
        for c in range(N_CORES):
            lo = row_off + c * rows_core
            arr = results[c]["out"]
            anchors[lo : lo + rows_core] = arr[:, col : col + N].reshape(-1, 4)
            xyxy[lo : lo + rows_core] = arr[:, col + N : col + 2 * N].reshape(
                -1, 4
            )
        col += 2 * N
        row_off += H * W * NUM_ANCHORS
    return anchors, xyxy


def kernel(feat3=None, feat4=None, feat5=None, feat6=None, **_ignored):
    # Outputs depend only on the (fixed) feature shapes; values are unused.
    res = _run(trace=False)
    return _assemble(res.results)


def _ensure_ntff_hook():
    """Inject antenv.axon_hooks (absent in this image) so that
    run_bass_kernel_spmd(trace=True) can reach the ctypes NTFF profiler."""
    import sys
    import types

    try:
        from antenv.axon_hooks import get_axon_ntff_profile_hook  # noqa: F401

        return
    except ImportError:
        pass
    import antenv
    from trn_agent_boot.trn_boot import _ntff_profile_via_ctypes

    hook = _ntff_profile_via_ctypes("/opt/axon/libaxon_pjrt.so")
    mod = types.ModuleType("antenv.axon_hooks")
    mod._hook = hook
    mod.set_axon_ntff_profile_hook = lambda h: setattr(mod, "_hook", h)
    mod.get_axon_ntff_profile_hook = lambda: mod._hook
    sys.modules["antenv.axon_hooks"] = mod
    antenv.axon_hooks = mod


def kernel_traced(**trace_kw):
    """Run with NTFF profiling; returns ((anchors, xyxy), BassKernelResults)."""
    _ensure_ntff_hook()
    from concourse import bass_utils

    bass_utils.upload_artifacts = lambda tmpdir: tmpdir  # no egress in sandbox
    res = _run(trace=True, **trace_kw)
    return _assemble(res.results), res


# revision 26
# speedup vs baseline: 1.2559x; 1.1841x over previous
"""Anchor-grid generation kernel for Trainium2 (8 NeuronCores, SPMD).

The reference computes RetinaNet-style anchors from the *shapes* of four FPN
feature maps — the feature values are never read.  Output is a tuple
(anchors_cxcywh, anchors_xyxy), each (783360, 4) float32.

Strategy: each anchor value decomposes as a rank-4 outer product over
(partition, free) once a level's per-core slice is laid out flat as
(128, N) on SBUF:

    tile[p, j] = 1*PHI[j] + cx0[p]*M0[j] + cy[p]*M1[j] + 1*WH[j]

which is exactly a K=4 fp32 matmul (lhsT = [1, cx0, cy, 1] per partition,
rhs = [PHI, M0, M1, WH]).  PE generates every output tile into PSUM,
ACT/DVE copy PSUM->SBUF, and DMA writes fully-contiguous DRAM ranges.
The 680 MiB of feature data never touches the device; per-core DRAM
traffic is ~3 MiB of pure output writes plus ~130 KB of seed constants.

All grid coordinates are exact fp32 integers, so the result is bit-exact
vs the fp32 reference (verified vs jax on CPU).
"""

import numpy as np

N_CORES = 8
# (H, W, stride, box_size) per pyramid level; hardcoded from the problem spec.
LEVELS = [(256, 256, 8, 32), (128, 128, 16, 64), (64, 64, 32, 128), (32, 32, 64, 256)]
NUM_ANCHORS = 9
TOTAL_ROWS = sum(H * W * NUM_ANCHORS for H, W, _, _ in LEVELS)  # 783360

# Per-level flat per-core layout: rows_per_core y-rows, g row-segments per
# y-row of Wseg grid columns each -> (128, N=Wseg*36) tile per core.
_LAYOUT = []
for H, W, S, B in LEVELS:
    rows_pc = H // N_CORES
    g = 128 // rows_pc
    Wseg = W // g
    _LAYOUT.append((H, W, S, B, rows_pc, g, Wseg, Wseg * 36))
# tensor names per (output, level)
_NAMES = [f"{t}{lvl}" for t in ("a", "x") for lvl in (3, 4, 5, 6)]


def _box_wh(box_size):
    # replicate reference._box_wh exactly (float64 math, fp32 cast at end)
    ratios = np.array([0.5, 1.0, 2.0])
    scales = np.array([2 ** 0.0, 2 ** (1.0 / 3.0), 2 ** (2.0 / 3.0)])
    scale_rep = np.tile(scales, len(ratios))
    ratio_rep = np.repeat(ratios, len(scales))
    side = box_size * scale_rep
    w = side * np.sqrt(ratio_rep)
    h = w / ratio_rep
    return np.stack([w, h], axis=-1).astype(np.float32)  # (9, 2)


def _host_consts():
    """K=5 fp16 seeds.  Every grid value (<=2044) is an exact fp16 integer;
    the irrational w/h templates are split hi+lo so the fp32 PSUM sum
    reconstructs them to ~1e-7 relative.

    Returns rhs_levels: per level a (5, 2*N) fp16 block [anchors | xyxy];
    lhst[c]: (5, 4*128) fp16 per core, rows = [1, cx0, cy, 1, 1]."""
    rhs_levels = []
    for H, W, S, B, rows_pc, g, Wseg, N in _LAYOUT:
        wh = _box_wh(B)
        a = np.arange(Wseg)
        k = np.arange(NUM_ANCHORS)
        base = (a[:, None] * 36 + k[None, :] * 4).ravel()  # (Wseg*9,)
        phi_v = np.repeat((S * a).astype(np.float32), NUM_ANCHORS)

        def hi_lo(full):
            hi = full.astype(np.float16)
            lo = (full - hi.astype(np.float32)).astype(np.float16)
            return hi, lo

        PHI = np.zeros(N, np.float16)
        M0 = np.zeros(N, np.float16)
        M1 = np.zeros(N, np.float16)
        WH = np.zeros(N, np.float32)
        PHI[base] = phi_v
        M0[base] = 1.0
        M1[base + 1] = 1.0
        WH[base + 2] = np.tile(wh[:, 0], Wseg)
        WH[base + 3] = np.tile(wh[:, 1], Wseg)
        WHhi, WHlo = hi_lo(WH)
        rhs_a = np.stack([PHI, M0, M1, WHhi, WHlo])

        PHIX = np.zeros(N, np.float16)
        M0X = np.zeros(N, np.float16)
        M1X = np.zeros(N, np.float16)
        WHX = np.zeros(N, np.float32)
        PHIX[base] = phi_v
        PHIX[base + 2] = phi_v
        M0X[base] = 1.0
        M0X[base + 2] = 1.0
        M1X[base + 1] = 1.0
        M1X[base + 3] = 1.0
        WHX[base] = np.tile(-wh[:, 0] / 2.0, Wseg)
        WHX[base + 1] = np.tile(-wh[:, 1] / 2.0, Wseg)
        WHX[base + 2] = np.tile(wh[:, 0] / 2.0, Wseg)
        WHX[base + 3] = np.tile(wh[:, 1] / 2.0, Wseg)
        WHXhi, WHXlo = hi_lo(WHX)
        rhs_x = np.stack([PHIX, M0X, M1X, WHXhi, WHXlo])
        rhs_levels.append(np.concatenate([rhs_a, rhs_x], axis=1))  # (5, 2N)

    lhst_per_core = []
    p = np.arange(128)
    for c in range(N_CORES):
        cols = []
        for H, W, S, B, rows_pc, g, Wseg, N in _LAYOUT:
            y = c * rows_pc + p // g
            cx0 = ((p % g) * Wseg + 0.5) * S
            cy = (y + 0.5) * S
            ones = np.ones(128, np.float16)
            cols.append(
                np.stack(
                    [ones, cx0.astype(np.float16), cy.astype(np.float16),
                     ones, ones]
                )
            )
        lhst_per_core.append(np.concatenate(cols, axis=1))  # (5, 512)
    return rhs_levels, lhst_per_core


def _build():
    """Raw bacc kernel (no TileContext): explicit engine blocks + semaphores.
    Avoids Tile's multi-microsecond end-of-kernel drain/barrier cascade.

    Dataflow per core: 1 input DMA (66 KB fp16 seed blob) -> 18 K=5 fp16
    matmuls into 8 PSUM banks -> PSUM->SBUF copies alternating DVE/ACT ->
    18 per-chunk HWDGE output DMAs (~3 MiB, the write roofline)."""
    import concourse.bacc as bacc
    import concourse.mybir as mybir

    f32 = mybir.dt.float32
    f16 = mybir.dt.float16
    rhs_np, lhst_np = _host_consts()

    # consts blob: [lhst(512) | r3 | r4 | r5 | r6] along free dim, 5 rows
    level_off = []
    off = 4 * 128
    for li in range(4):
        level_off.append(off)
        off += rhs_np[li].shape[1]
    blob_w = off  # 6632

    # chunk plan: (name, level, c0, c1, blob col offset).  The very first
    # chunk is 256 wide so the first PSUM->SBUF copy (and with it the output
    # DMA stream) starts as early as possible.
    chunks = []
    for li, (H, W, S, B, rows_pc, g, Wseg, N) in enumerate(_LAYOUT):
        for ti, t in enumerate(("a", "x")):
            if li == 0 and ti == 0:
                bounds = [0, 256, 768, 1280, 1792, 2304]
            else:
                bounds = list(range(0, N, 512)) + [N]
            for c0, c1 in zip(bounds[:-1], bounds[1:]):
                chunks.append(
                    (f"{t}{li + 3}", li, c0, c1, level_off[li] + ti * N + c0)
                )
    n_chunks = len(chunks)  # 18

    nc = bacc.Bacc("TRN2", target_bir_lowering=False, debug=False,
                   num_devices=N_CORES)

    # One fused output: columns [a3|x3|a4|x4|a5|x5|a6|x6] in chunk order —
    # both the SBUF staging tensor and the DRAM output share this layout, so
    # any run of consecutive chunks is a single rectangular DMA.
    out_w = sum(c1 - c0 for (_, _, c0, c1, _) in chunks)  # 6120
    chunk_col = []
    _cc = 0
    for _, _, c0, c1, _ in chunks:
        chunk_col.append(_cc)
        _cc += c1 - c0

    consts_in = nc.dram_tensor("consts", [5, blob_w], f16, kind="ExternalInput")
    out_dram = nc.dram_tensor("out", [128, out_w], f32, kind="ExternalOutput")

    consts = nc.alloc_sbuf_tensor("consts_sb", [5, blob_w], f16)
    stage = nc.alloc_sbuf_tensor("stage", [128, out_w], f32)
    psum = [nc.alloc_psum_tensor(f"ps{b}", [128, 512], f32) for b in range(8)]

    # cumulative per-copy-engine counts after chunk j (engine = j % 2)
    cnt_at = []
    _c = [0, 0]
    for j in range(n_chunks):
        _c[j % 2] += 1
        cnt_at.append(tuple(_c))

    # output DMA batches (by chunk index).  Fewer DMA instructions is
    # faster in itself (the NEFF teardown costs ~0.8us per DMA), but each
    # batch can only issue after its last chunk is copied, so front-load:
    # tiny first batch for an early stream start, big middle, small tail.
    batches = [(0, 0), (1, 1), (2, 3), (4, 7), (8, 12), (13, 17)]

    with (
        nc.Block() as block,
        nc.semaphore("in_sem") as in_sem,
        nc.semaphore("in_sem2") as in_sem2,
        nc.semaphore("mm_sem") as mm_sem,
        nc.semaphore("cp0") as cp0,
        nc.semaphore("cp1") as cp1,
        nc.semaphore("od") as od,
    ):
        cp = (cp0, cp1)

        @block.sync
        def _(sync):
            # split input load: part 1 (lhsT + all of a3's rhs) lands first
            # so MM0 starts one small-DMA-receipt earlier; part 2's receipt
            # latency hides behind the five a3 chunks
            sync.dma_start(
                consts[:, 0:2816], consts_in[:, 0:2816]
            ).then_inc(in_sem, 16)
            sync.dma_start(
                consts[:, 2816:blob_w], consts_in[:, 2816:blob_w]
            ).then_inc(in_sem2, 16)
            for jlo, jhi in batches:
                sync.wait_ge(cp0, cnt_at[jhi][0])
                if cnt_at[jhi][1]:
                    sync.wait_ge(cp1, cnt_at[jhi][1])
                lo = chunk_col[jlo]
                hi = chunk_col[jhi] + (chunks[jhi][3] - chunks[jhi][2])
                sync.dma_start(out_dram[:, lo:hi], stage[:, lo:hi]).then_inc(
                    od, 16
                )
            if not _SKIP_OD_WAIT:
                sync.wait_ge(od, 16 * len(batches))

        @block.tensor
        def _(pe):
            pe.wait_ge(in_sem, 16)
            for j, (name, li, c0, c1, bcol) in enumerate(chunks):
                if j == 5:
                    pe.wait_ge(in_sem2, 16)
                if j >= 8:
                    f = j - 8  # chunk whose copy frees this bank
                    pe.wait_ge(cp[f % 2], cnt_at[f][f % 2])
                n = c1 - c0
                pe.matmul(
                    psum[j % 8][:, :n],
                    consts[:, li * 128 : (li + 1) * 128],
                    consts[:, bcol : bcol + n],
                    start=True,
                    stop=True,
                ).then_inc(mm_sem, 1)

        @block.vector
        def _(dve):
            for j, (name, li, c0, c1, _) in enumerate(chunks):
                if j % 2 != 0:
                    continue
                dve.wait_ge(mm_sem, j + 1)
                n = c1 - c0
                col = chunk_col[j]
                dve.tensor_copy(
                    stage[:, col : col + n], psum[j % 8][:, :n]
                ).then_inc(cp0, 1)

        @block.scalar
        def _(act):
            for j, (name, li, c0, c1, _) in enumerate(chunks):
                if j % 2 != 1:
                    continue
                act.wait_ge(mm_sem, j + 1)
                n = c1 - c0
                col = chunk_col[j]
                act.copy(stage[:, col : col + n], psum[j % 8][:, :n]).then_inc(
                    cp1, 1
                )

    nc.compile()
    blob = [
        np.concatenate([lhst_np[c]] + rhs_np, axis=1).astype(np.float16)
        for c in range(N_CORES)
    ]
    in_maps = [{"consts": blob[c]} for c in range(N_CORES)]
    return nc, in_maps


def _build_tile():
    import concourse.bass as bass
    import concourse.bacc as bacc
    import concourse.mybir as mybir
    from concourse import tile

    f32 = mybir.dt.float32
    f16 = mybir.dt.float16
    rhs_np, lhst_np = _host_consts()

    nc = bacc.Bacc("TRN2", target_bir_lowering=False, debug=False,
                   num_devices=N_CORES)

    lhst_in = nc.dram_tensor("lhst", [5, 4 * 128], f16, kind="ExternalInput")
    rhs_in = [
        nc.dram_tensor(f"r{li + 3}", list(rhs_np[li].shape), f16,
                       kind="ExternalInput")
        for li in range(4)
    ]
    outs = {}
    for t in ("a", "x"):
        for li, (H, W, S, B, rows_pc, g, Wseg, N) in enumerate(_LAYOUT):
            outs[f"{t}{li + 3}"] = nc.dram_tensor(
                f"{t}{li + 3}", [128, N], f32, kind="ExternalOutput"
            )

    with tile.TileContext(nc) as tc:
        with (
            tc.tile_pool(name="consts", bufs=1) as cpool,
            tc.tile_pool(name="stage", bufs=1) as spool,
            tc.tile_pool(name="psum", bufs=8, space="PSUM") as ppool,
        ):
            lhst = cpool.tile([5, 4 * 128], f16)
            nc.sync.dma_start(lhst[:], lhst_in[:])
            rhs = []
            for li in range(4):
                r = cpool.tile(list(rhs_np[li].shape), f16, tag=f"rhs{li}")
                nc.sync.dma_start(r[:], rhs_in[li][:])
                rhs.append(r)

            copy_i = 0
            for li, (H, W, S, B, rows_pc, g, Wseg, N) in enumerate(_LAYOUT):
                lhsT = lhst[:, li * 128 : (li + 1) * 128]
                for ti, t in enumerate(("a", "x")):
                    name = f"{t}{li + 3}"
                    out_t = spool.tile([128, N], f32, tag=name)
                    for c0 in range(0, N, 512):
                        c1 = min(c0 + 512, N)
                        ps = ppool.tile([128, c1 - c0], f32)
                        nc.tensor.matmul(
                            ps[:],
                            lhsT,
                            rhs[li][:, ti * N + c0 : ti * N + c1],
                            start=True,
                            stop=True,
                        )
                        if copy_i % 2 == 0:
                            nc.vector.tensor_copy(out_t[:, c0:c1], ps[:])
                        else:
                            nc.scalar.copy(out_t[:, c0:c1], ps[:])
                        copy_i += 1
                    nc.sync.dma_start(outs[name][:], out_t[:])

    nc.compile()
    in_maps = [
        {"lhst": lhst_np[c], **{f"r{li + 3}": rhs_np[li] for li in range(4)}}
        for c in range(N_CORES)
    ]
    return nc, in_maps


# Rely on the NEFF epilogue's DMA-queue quiesce instead of an explicit
# completion wait on the output DMAs (saves the ~1.7us HBM-write sem receipt
# and lets the fixed epilogue overlap the stream tail).
_SKIP_OD_WAIT = True

_CACHE = None


def _run(trace=False, **trace_kw):
    global _CACHE
    if _CACHE is None:
        _CACHE = _build()
    nc, in_maps = _CACHE
    from concourse.bass_utils import run_bass_kernel_spmd

    return run_bass_kernel_spmd(
        nc, in_maps, core_ids=list(range(N_CORES)), trace=trace, **trace_kw
    )


def _assemble(results):
    """results[c]["out"] is (128, 6120) fused as [a3|x3|a4|x4|a5|x5|a6|x6]."""
    anchors = np.empty((TOTAL_ROWS, 4), np.float32)
    xyxy = np.empty((TOTAL_ROWS, 4), np.float32)
    row_off = 0
    col = 0
    for li, (H, W, S, B, rows_pc, g, Wseg, N) in enumerate(_LAYOUT):
        rows_core = 128 * N // 4
        for c in range(N_CORES):
            lo = row_off + c * rows_core
            arr = results[c]["out"]
            anchors[lo : lo + rows_core] = arr[:, col : col + N].reshape(-1, 4)
            xyxy[lo : lo + rows_core] = arr[:, col + N : col + 2 * N].reshape(
                -1, 4
            )
        col += 2 * N
        row_off += H * W * NUM_ANCHORS
    return anchors, xyxy


def kernel(feat3=None, feat4=None, feat5=None, feat6=None, **_ignored):
    # Outputs depend only on the (fixed) feature shapes; values are unused.
    res = _run(trace=False)
    return _assemble(res.results)


def _ensure_ntff_hook():
    """Inject antenv.axon_hooks (absent in this image) so that
    run_bass_kernel_spmd(trace=True) can reach the ctypes NTFF profiler."""
    import sys
    import types

    try:
        from antenv.axon_hooks import get_axon_ntff_profile_hook  # noqa: F401

        return
    except ImportError:
        pass
    import antenv
    from trn_agent_boot.trn_boot import _ntff_profile_via_ctypes

    hook = _ntff_profile_via_ctypes("/opt/axon/libaxon_pjrt.so")
    mod = types.ModuleType("antenv.axon_hooks")
    mod._hook = hook
    mod.set_axon_ntff_profile_hook = lambda h: setattr(mod, "_hook", h)
    mod.get_axon_ntff_profile_hook = lambda: mod._hook
    sys.modules["antenv.axon_hooks"] = mod
    antenv.axon_hooks = mod


def kernel_traced(**trace_kw):
    """Run with NTFF profiling; returns ((anchors, xyxy), BassKernelResults)."""
    _ensure_ntff_hook()
    from concourse import bass_utils

    bass_utils.upload_artifacts = lambda tmpdir: tmpdir  # no egress in sandbox
    res = _run(trace=True, **trace_kw)
    return _assemble(res.results), res
